# revision 1
# baseline (speedup 1.0000x reference)
"""Causal self-attention (B=4, T=2048, C=1024, 16 heads) on 8 TRN2 NeuronCores.

Sharding: tensor-parallel over heads. Each core owns 2 heads (128 of the
1024 q/k/v dims): wq/wk/wv are split by rows (output dim), wo by columns.
Each core computes a full [C, B*T] partial of the output projection; the
host sums the 8 partials.

On-core layout is "transposed": activations live as [feature, token] so
every matmul has tokens on the moving free dim (>=256 wide -> float32r
matmuls run at 1 cycle/row). Attention is computed as s^T = K Q^T with
keys on partitions; softmax max-subtraction is skipped (logits are O(10),
exp is safe in fp32) and the denominator comes from a ones-column
appended to V in the P^T @ V matmul. Causal masking replaces masked
probabilities with exp(-10) (the module masks logits with -10, not -inf).
Probabilities and V run in bf16 (denominator and numerator use the same
quantized probs, so the bias largely cancels); projections and scores
stay in f32r.
"""

import os
import sys

import numpy as np

for _p in ("/opt/trn_rl_repo",):
    if _p not in sys.path and os.path.isdir(_p):
        sys.path.insert(0, _p)

_B, _T, _C = 4, 2048, 1024
_NHEAD, _HD = 16, 64
_NC = 8
_LOC = (_NHEAD // _NC) * _HD  # feature dims per core = 128 (2 heads)
_BT = _B * _T                 # 8192 tokens
_TC = 512                     # token chunk (psum bank / moving-operand width)
_NTC = _BT // _TC             # 16 projection chunks
_KC = _C // 128               # 8 contraction chunks over the embedding
_NQC = _T // _TC              # 4 query chunks per batch
_NKB = _T // 128              # 16 key blocks per batch
_EXPM = float(np.exp(-10.0))  # exp of the mask fill value

TRACE = bool(int(os.environ.get("KERNEL_TRACE", "0")))
LAST_EXEC_NS = None
LAST_RESULTS = None

_cache = {}


def _build():
    import concourse.mybir as mybir
    import concourse.tile as tile
    from concourse import bacc

    f32 = mybir.dt.float32
    f32r = mybir.dt.float32r
    bf16 = mybir.dt.bfloat16
    AF = mybir.ActivationFunctionType

    nc = bacc.Bacc("TRN2", target_bir_lowering=False, debug=False)

    xT_d = nc.dram_tensor("xT", [_C, _BT], f32r, kind="ExternalInput").ap()
    wqT_d = nc.dram_tensor("wqT", [_C, _LOC], f32r, kind="ExternalInput").ap()
    wkT_d = nc.dram_tensor("wkT", [_C, _LOC], f32r, kind="ExternalInput").ap()
    wvT_d = nc.dram_tensor("wvT", [_C, _LOC], f32r, kind="ExternalInput").ap()
    woT_d = nc.dram_tensor("woT", [_LOC, _C], f32r, kind="ExternalInput").ap()
    idc_d = nc.dram_tensor("identc", [128, 64], f32r, kind="ExternalInput").ap()
    oneb_d = nc.dram_tensor("onesb", [128, 1], bf16, kind="ExternalInput").ap()
    oner_d = nc.dram_tensor("onesr", [1, 64], f32r, kind="ExternalInput").ap()
    outT_d = nc.dram_tensor("outT", [_C, _BT], f32, kind="ExternalOutput").ap()

    xT_v = xT_d.rearrange("(c p) n -> p c n", p=128)    # [128, 8, 8192]
    wq_v = wqT_d.rearrange("(c p) m -> p c m", p=128)   # [128, 8, 128]
    wk_v = wkT_d.rearrange("(c p) m -> p c m", p=128)
    wv_v = wvT_d.rearrange("(c p) m -> p c m", p=128)
    wo_v = woT_d.rearrange("p (m n) -> p m n", n=128)   # [128, 8, 128]

    with tile.TileContext(nc) as tc:
        with (
            tc.tile_pool(name="consts", bufs=1) as cp,
            tc.tile_pool(name="sb", bufs=2) as sp,
            tc.tile_pool(name="ps", bufs=2, space="PSUM") as pp,
        ):
            # first token chunk of x starts loading before anything else
            xa0 = sp.tile([128, 4, _TC], f32r, tag="xa", bufs=2)
            xb0 = sp.tile([128, 4, _TC], f32r, tag="xb", bufs=2)
            for c4 in range(4):
                nc.sync.dma_start(xa0[:, c4, :], xT_v[:, c4, 0:_TC])
                nc.sync.dma_start(xb0[:, c4, :], xT_v[:, 4 + c4, 0:_TC])
            w_sb = {}
            for nm, v in (("q", wq_v), ("k", wk_v), ("v", wv_v)):
                t = cp.tile([128, _KC, 128], f32r, tag=f"w{nm}")
                nc.sync.dma_start(t[:], v[:])
                w_sb[nm] = t
            wo_sb = cp.tile([128, _KC, 128], f32r, tag="wo")
            nc.sync.dma_start(wo_sb[:], wo_v[:])
            ident = cp.tile([128, 64], f32r, tag="ident")
            nc.sync.dma_start(ident[:], idc_d[:])
            ones1 = cp.tile([65, 64], f32r, tag="ones1")
            nc.sync.dma_start(ones1[64:65, :], oner_d[:])
            ones_b = cp.tile([128, 1], bf16, tag="onesb")
            nc.sync.dma_start(ones_b[:], oneb_d[:])

            qT = cp.tile([128, _BT], f32r, tag="qT")
            kT = cp.tile([128, _BT], f32r, tag="kT")
            # v in [token, dim] layout per 128-token block, per head, with a
            # trailing ones column (row sums -> softmax denominator)
            vaug = cp.tile([128, _BT // 128, 2, 65], bf16, tag="vaug")
            for h in range(2):
                nc.sync.dma_start(
                    vaug[:, :, h, 64:65],
                    oneb_d[:, 0:1].to_broadcast([128, _BT // 128, 1]),
                )

            # ---------------- q/k/v projections ----------------
            for t in range(_NTC):
                tok = slice(t * _TC, (t + 1) * _TC)
                if t == 0:
                    halves = (xa0, xb0)
                else:
                    xa = sp.tile([128, 4, _TC], f32r, tag="xa", bufs=2)
                    xb = sp.tile([128, 4, _TC], f32r, tag="xb", bufs=2)
                    for c4 in range(4):
                        nc.sync.dma_start(xa[:, c4, :], xT_v[:, c4, tok])
                        nc.sync.dma_start(xb[:, c4, :], xT_v[:, 4 + c4, tok])
                    halves = (xa, xb)
                for nm in ("q", "k", "v"):
                    ps = pp.tile([128, _TC], f32, tag="psC", bufs=2)
                    for c in range(_KC):
                        nc.tensor.matmul(
                            ps[:],
                            w_sb[nm][:, c, :],
                            halves[c // 4][:, c % 4, :],
                            start=(c == 0),
                            stop=(c == _KC - 1),
                        )
                    if nm == "q":
                        nc.vector.tensor_copy(qT[:, tok], ps[:])
                    elif nm == "k":
                        nc.vector.tensor_copy(kT[:, tok], ps[:])
                    else:
                        vtc = sp.tile([128, _TC], f32r, tag="vtc", bufs=2)
                        nc.vector.tensor_copy(vtc[:], ps[:])
                        for h in range(2):
                            tp = pp.tile([128, 4, 64], f32, tag="psC",
                                         bufs=2)
                            for s4 in range(4):
                                nc.tensor.transpose(
                                    tp[:, s4, :].bitcast(f32r),
                                    vtc[h * 64:(h + 1) * 64,
                                        s4 * 128:(s4 + 1) * 128],
                                    ident[h * 64:(h + 1) * 64, :],
                                )
                            nc.vector.tensor_copy(
                                vaug[:, t * 4:t * 4 + 4, h, 0:64], tp[:]
                            )

            # ---------------- attention + output projection ----------------
            for b in range(_B):
                ycat = sp.tile([128, _T], f32r, tag="ycat", bufs=2)
                for h in range(2):
                    rows = slice(h * 64, (h + 1) * 64)
                    ytmp = sp.tile([65, _T], f32r, tag="ytmp", bufs=2)
                    # column sums of v over each chunk's fully-masked key
                    # blocks, accumulated in PSUM: suf[:, c] = sum over
                    # kb >= 4c+4 of (v_kb^T @ 1).  Applied (scaled by
                    # exp(-10)) as a bias when copying y out of PSUM.
                    suf_ps = pp.tile([65, _NQC - 1], f32, tag="suf", bufs=1)
                    for c in range(_NQC - 1):
                        for kb in range(4 * c + 4, _NKB):
                            nc.tensor.matmul(
                                suf_ps[:, c:c + 1],
                                vaug[:, b * 16 + kb, h, :],
                                ones_b[:],
                                start=(kb == 4 * c + 4),
                                stop=(kb == _NKB - 1),
                            )
                    suf_sb = sp.tile([65, _NQC - 1], f32, tag="suf", bufs=2)
                    nc.scalar.activation(
                        suf_sb[:], suf_ps[:], AF.Copy, scale=_EXPM
                    )
                    for c in range(_NQC):
                        qc = slice(b * _T + c * _TC, b * _T + (c + 1) * _TC)
                        cc = slice(c * _TC, (c + 1) * _TC)
                        yps = pp.tile([65, _TC], f32, tag="yT", bufs=1)
                        for kb in range(4 * c + 4):
                            j = kb - 4 * c
                            sps = pp.tile([128, _TC], f32, tag="psB",
                                          bufs=4)
                            # band blocks j=1,2: the leading 128j columns
                            # are fully masked, so the score matmul only
                            # needs the tail (tail >= 256 keeps f32r fast)
                            off = 128 * j if j in (1, 2) else 0
                            nc.tensor.matmul(
                                sps[:, off:],
                                kT[rows,
                                   b * _T + kb * 128:
                                   b * _T + (kb + 1) * 128],
                                qT[rows,
                                   b * _T + c * _TC + off:
                                   b * _T + (c + 1) * _TC],
                                start=True, stop=True,
                            )
                            pexp = sp.tile([128, _TC], bf16, tag="pexp",
                                           bufs=16)
                            if j >= 1:
                                # leading 128j columns are fully masked;
                                # the affine_select fills them below
                                nc.scalar.activation(
                                    pexp[:, 128 * j:], sps[:, 128 * j:],
                                    AF.Exp, scale=0.125
                                )
                            else:
                                nc.scalar.activation(
                                    pexp[:], sps[:], AF.Exp, scale=0.125
                                )
                            if j >= 0:
                                # causal: keep where qi - ki - 128j >= 0,
                                # else fill exp(-10); columns right of the
                                # diagonal strip are always valid
                                w = 128 * (j + 1)
                                nc.gpsimd.affine_select(
                                    out=pexp[:, 0:w],
                                    in_=pexp[:, 0:w],
                                    compare_op=mybir.AluOpType.is_ge,
                                    fill=_EXPM,
                                    base=-128 * j,
                                    pattern=[[1, w]],
                                    channel_multiplier=-1,
                                )
                            nc.tensor.matmul(
                                yps[:],
                                vaug[:, b * 16 + kb, h, :],
                                pexp[:],
                                start=(kb == 0),
                                stop=(kb == 4 * c + 3),
                            )
                        if c < _NQC - 1:
                            nc.scalar.activation(
                                ytmp[:, cc], yps[:], AF.Identity,
                                bias=suf_sb[:, c:c + 1],
                            )
                        else:
                            nc.scalar.copy(ytmp[:, cc], yps[:])
                    # normalize: row 64 holds the softmax denominator;
                    # broadcast Z over the 64 dims via a K=1 matmul, then
                    # reciprocal + multiply per chunk
                    for c in range(_NQC):
                        cc = slice(c * _TC, (c + 1) * _TC)
                        zps = pp.tile([64, _TC], f32, tag="psC", bufs=2)
                        nc.tensor.matmul(
                            zps[:],
                            ones1[64:65, :],
                            ytmp[64:65, cc],
                            start=True, stop=True,
                        )
                        zrec = sp.tile([64, _TC], f32, tag="zrec", bufs=2)
                        nc.vector.reciprocal(zrec[:], zps[:])
                        nc.vector.tensor_mul(
                            ycat[rows, cc], ytmp[0:64, cc], zrec[:]
                        )
                # last batch: chunk-outer order starts the output drain
                # as soon as each ycat chunk is normalized
                if b == _B - 1:
                    mc2 = [(m, c2) for c2 in range(_NQC)
                           for m in range(_KC)]
                else:
                    mc2 = [(m, c2) for m in range(_KC)
                           for c2 in range(_NQC)]
                for m, c2 in mc2:
                    ops = pp.tile([128, _TC], f32, tag="psC", bufs=2)
                    nc.tensor.matmul(
                        ops[:],
                        wo_sb[:, m, :],
                        ycat[:, c2 * _TC:(c2 + 1) * _TC],
                        start=True, stop=True,
                    )
                    ostg = sp.tile([128, _TC], f32, tag="ostg", bufs=6)
                    if b == _B - 1 and c2 % 2 == 0:
                        nc.scalar.copy(ostg[:], ops[:])
                    else:
                        nc.vector.tensor_copy(ostg[:], ops[:])
                    nc.sync.dma_start(
                        outT_d[m * 128:(m + 1) * 128,
                               b * _T + c2 * _TC:b * _T + (c2 + 1) * _TC],
                        ostg[:],
                    )

    nc.compile()
    return nc, outT_d.name


def _get_nc():
    if "nc" not in _cache:
        _cache["nc"] = _build()
    return _cache["nc"]


def kernel(**inputs):
    import ml_dtypes

    from concourse.bass_utils import run_bass_kernel_spmd

    x = np.ascontiguousarray(np.asarray(inputs["x"]), dtype=np.float32)
    wq = np.ascontiguousarray(np.asarray(inputs["wq"]), dtype=np.float32)
    wk = np.ascontiguousarray(np.asarray(inputs["wk"]), dtype=np.float32)
    wv = np.ascontiguousarray(np.asarray(inputs["wv"]), dtype=np.float32)
    wo = np.ascontiguousarray(np.asarray(inputs["wo"]), dtype=np.float32)

    xT = np.ascontiguousarray(x.reshape(_BT, _C).T)
    identc = np.zeros((128, 64), dtype=np.float32)
    identc[np.arange(128), np.arange(128) % 64] = 1.0
    onesb = np.ones((128, 1), dtype=ml_dtypes.bfloat16)
    onesr = np.ones((1, 64), dtype=np.float32)

    in_maps = []
    for i in range(_NC):
        r = slice(_LOC * i, _LOC * (i + 1))
        in_maps.append({
            "xT": xT,
            "wqT": np.ascontiguousarray(wq[r].T),
            "wkT": np.ascontiguousarray(wk[r].T),
            "wvT": np.ascontiguousarray(wv[r].T),
            "woT": np.ascontiguousarray(wo[:, r].T),
            "identc": identc,
            "onesb": onesb,
            "onesr": onesr,
        })

    nc, outname = _get_nc()
    try:
        res = run_bass_kernel_spmd(nc, in_maps, list(range(_NC)), trace=TRACE)
    except ModuleNotFoundError:
        # NTFF profiling hook unavailable in this container
        res = run_bass_kernel_spmd(nc, in_maps, list(range(_NC)), trace=False)

    global LAST_EXEC_NS, LAST_RESULTS
    LAST_EXEC_NS = res.exec_time_ns
    LAST_RESULTS = res

    acc = np.zeros((_C, _BT), dtype=np.float64)
    for i in range(_NC):
        acc += res.results[i][outname]
    return np.ascontiguousarray(acc.T).reshape(_B, _T, _C).astype(np.float32)



# revision 28
# speedup vs baseline: 1.1651x; 1.1651x over previous
"""Causal self-attention (B=4, T=2048, C=1024, 16 heads) on 8 TRN2 NeuronCores.

Sharding: tensor-parallel over heads. Each core owns 2 heads (128 of the
1024 q/k/v dims): wq/wk/wv are split by rows (output dim), wo by columns.
Each core computes a full [C, B*T] partial of the output projection; the
host sums the 8 partials.

Design notes (v2):
- All projections and scores run in bf16 (1 cycle/row at any moving width).
- Attention-weight matmuls for fully-valid ("full") key blocks run in
  fp8e4m3 with DoubleRow perf mode: pairs of key blocks form the two
  k-tiles of one matmul, halving PE occupancy. The band blocks (which
  contain the causal diagonal, where logits are large ~|q|^2/8 and the
  exp(-10) mask fill is below fp8 range) stay in bf16 and only compute the
  un-masked tail of each 128-query group.
- The attention stationary is [keys, 128] with v dims in cols 0-63 and
  ones in cols 64-127, so PSUM rows 64-127 replicate the softmax
  denominator Z: normalization needs no broadcast matmul.
- Masked contributions (exp(-10) terms the module keeps in its softmax)
  are added exactly via per-query-block suffix sums of V-block column
  sums, applied as per-partition scalars while copying PSUM out.
- exp is scaled by e^-1 so fp8 probabilities sit inside fp8e4m3 range;
  the bias cancels between numerator and denominator.
- Output partials are written as bf16 (halves the drain DMA) and summed
  in float64 on the host.
"""

import os
import sys

import numpy as np

for _p in ("/opt/trn_rl_repo",):
    if _p not in sys.path and os.path.isdir(_p):
        sys.path.insert(0, _p)

_B, _T, _C = 4, 2048, 1024
_NHEAD, _HD = 16, 64
_NC = 8
_LOC = (_NHEAD // _NC) * _HD  # feature dims per core = 128 (2 heads)
_BT = _B * _T                 # 8192 tokens
_TC = 512                     # token chunk (psum bank / moving width)
_NTC = _BT // _TC             # 16 projection chunks
_KC = _C // 128               # 8 contraction chunks over the embedding
_NQC = _T // _TC              # 4 query chunks per batch
_NKB = _T // 128              # 16 key blocks per batch
_EB = -3.5                    # exp bias: pexp = exp(0.125*s + _EB); keeps max full-block prob (e^8.54+_EB=154) under fp8e4m3 max 240
_EXPM = float(np.exp(-10.0 + _EB))  # mask-fill value after exp, biased

TRACE = bool(int(os.environ.get("KERNEL_TRACE", "0")))
V_NODR = bool(int(os.environ.get("V_NODR", "0")))
V_NOSEL = bool(int(os.environ.get("V_NOSEL", "0")))
V_NOBAND = bool(int(os.environ.get("V_NOBAND", "0")))
V_NOFULL = bool(int(os.environ.get("V_NOFULL", "0")))
V_NONORM = bool(int(os.environ.get("V_NONORM", "0")))
V_NOOP = bool(int(os.environ.get("V_NOOP", "0")))
V_NOVA = bool(int(os.environ.get("V_NOVA", "0")))
V_NOCS = bool(int(os.environ.get("V_NOCS", "0")))
V_NOTP = bool(int(os.environ.get("V_NOTP", "0")))
V_X4 = bool(int(os.environ.get("V_X4", "0")))
LAST_EXEC_NS = None
LAST_RESULTS = None

_cache = {}


def _build():
    import concourse.mybir as mybir
    import concourse.tile as tile
    from concourse import bacc

    f32 = mybir.dt.float32
    f32r = mybir.dt.float32r
    bf16 = mybir.dt.bfloat16
    fp8 = mybir.dt.float8e4
    AF = mybir.ActivationFunctionType
    DR = mybir.MatmulPerfMode.DoubleRow

    nc = bacc.Bacc("TRN2", target_bir_lowering=False, debug=False)

    xT_d = nc.dram_tensor("xT", [_C, _BT], bf16, kind="ExternalInput").ap()
    wqT_d = nc.dram_tensor("wqT", [_C, _LOC], bf16, kind="ExternalInput").ap()
    wkT_d = nc.dram_tensor("wkT", [_C, _LOC], bf16, kind="ExternalInput").ap()
    wvT_d = nc.dram_tensor("wvT", [_C, _LOC], bf16, kind="ExternalInput").ap()
    woT_d = nc.dram_tensor("woT", [_LOC, _C], bf16, kind="ExternalInput").ap()
    idc_d = nc.dram_tensor("identc", [128, 64], f32r, kind="ExternalInput").ap()
    oneb_d = nc.dram_tensor("onesb", [128, 1], bf16, kind="ExternalInput").ap()
    outT_d = nc.dram_tensor("outT", [_C, _BT], bf16, kind="ExternalOutput").ap()

    xT_v = xT_d.rearrange("(c p) n -> p c n", p=128)    # [128, 8, 8192]
    wq_v = wqT_d.rearrange("(c p) m -> p c m", p=128)   # [128, 8, 128]
    wk_v = wkT_d.rearrange("(c p) m -> p c m", p=128)
    wv_v = wvT_d.rearrange("(c p) m -> p c m", p=128)
    wo_v = woT_d.rearrange("p (m n) -> p m n", n=128)   # [128, 8, 128]

    with tile.TileContext(nc) as tc:
        with (
            tc.tile_pool(name="consts", bufs=1) as cp,
            tc.tile_pool(name="sb", bufs=2) as sp,
            tc.tile_pool(name="ps", bufs=2, space="PSUM") as pp,
        ):
            # ---------------- persistent SBUF tensors ----------------
            # q/k in bf16, [dims(2 heads x 64), {q,k}, tokens]
            qk = cp.tile([128, 2, _BT], bf16, tag="qk")
            # v stationary, banded: [keys, block, head, 128] with ones in
            # cols 64-127 (denominator rows)
            vab = cp.tile([128, _B * _NKB, 2, 128], bf16, tag="vab")
            # fp8 copy for DoubleRow pairs: [keys, pair, parity, head, 128]
            va8 = cp.tile([128, _B * _NKB // 2, 2, 2, 128], fp8, tag="va8")
            w_sb = {}
            for nm, v in (("q", wq_v), ("k", wk_v), ("v", wv_v)):
                t = cp.tile([128, _KC, 128], bf16, tag=f"w{nm}")
                nc.sync.dma_start(t[:], v[:])
                w_sb[nm] = t
            wo_sb = cp.tile([128, _KC, 128], bf16, tag="wo")
            nc.sync.dma_start(wo_sb[:], wo_v[:])
            ident = cp.tile([128, 64], f32r, tag="ident")
            nc.sync.dma_start(ident[:], idc_d[:])
            ones_b = cp.tile([128, 1], bf16, tag="onesb")
            nc.sync.dma_start(ones_b[:], oneb_d[:])
            ebias = cp.tile([128, 1], f32, tag="ebias")
            nc.vector.memset(ebias[:], _EB)

            from collections import deque
            filler = deque()

            def pop_filler(n=1):
                for _ in range(n):
                    if filler:
                        filler.popleft()()

            def drain_filler():
                while filler:
                    filler.popleft()()

            def load_x(t):
                xa = sp.tile([128, 4, _TC], bf16, tag="xa", bufs=3)
                xb = sp.tile([128, 4, _TC], bf16, tag="xb", bufs=3)
                tok = slice(t * _TC, (t + 1) * _TC)
                if V_X4:
                    for c4 in range(4):
                        nc.sync.dma_start(xa[:, c4, :], xT_v[:, c4, tok])
                        nc.sync.dma_start(xb[:, c4, :], xT_v[:, 4 + c4, tok])
                else:
                    nc.sync.dma_start(xa[:], xT_v[:, 0:4, tok])
                    nc.sync.dma_start(xb[:], xT_v[:, 4:8, tok])
                return (xa, xb)

            def emit_proj(t, colsum_sb):
                """Queue chunk t's projection work as filler closures."""
                halves = load_x(t)
                tok = slice(t * _TC, (t + 1) * _TC)
                st = {}

                def qk_mms(i, nm, lo):
                    def go():
                        key = f"ps{nm}"
                        if key not in st:
                            st[key] = pp.tile([128, _TC], f32,
                                              tag="ps1", bufs=6,
                                              name=f"ps{nm}{t}")
                        for c in range(lo, lo + 4):
                            nc.tensor.matmul(
                                st[key][:],
                                w_sb[nm][:, c, :],
                                halves[c // 4][:, c % 4, :],
                                start=(c == 0),
                                stop=(c == _KC - 1),
                            )
                    return go

                def q_copy():
                    nc.vector.tensor_copy(qk[:, 0, tok], st["psq"][:])

                def k_copy():
                    nc.vector.tensor_copy(qk[:, 1, tok], st["psk"][:])

                def v_mms(lo):
                    def go():
                        if "psv" not in st:
                            st["psv"] = pp.tile([128, _TC], f32,
                                                tag="ps1", bufs=6,
                                                name=f"psv{t}")
                        for c in range(lo, lo + 4):
                            nc.tensor.matmul(
                                st["psv"][:],
                                w_sb["v"][:, c, :],
                                halves[c // 4][:, c % 4, :],
                                start=(c == 0),
                                stop=(c == _KC - 1),
                            )
                    return go

                def v_copy():
                    if V_NOTP:
                        return
                    st["vtc"] = sp.tile([128, _TC], f32r, tag="vtc", bufs=3,
                                        name=f"vtc{t}")
                    nc.vector.tensor_copy(st["vtc"][:], st["psv"][:])

                def transposes(h, s0):
                    def go():
                        if V_NOTP:
                            return
                        key = f"tp{h}"
                        if key not in st:
                            st[key] = pp.tile([128, 4, 64], f32,
                                              tag="ps1", bufs=6,
                                              name=f"tp{t}_{h}")
                        for s4 in range(s0, s0 + 4):
                            nc.tensor.transpose(
                                st[key][:, s4, :].bitcast(f32r),
                                st["vtc"][h * 64:(h + 1) * 64,
                                          s4 * 128:(s4 + 1) * 128],
                                ident[h * 64:(h + 1) * 64, :],
                            )
                    return go

                def va_copies():
                    if V_NOVA:
                        return
                    for h in range(2):
                        nc.vector.tensor_copy(
                            vab[:, t * 4:t * 4 + 4, h, 0:64],
                            st[f"tp{h}"][:],
                        )
                        nc.vector.tensor_copy(
                            va8[:, t * 2:t * 2 + 2, :, h, 0:64],
                            st[f"tp{h}"][:],
                        )

                def colsums():
                    if V_NOCS:
                        return
                    sufp = pp.tile([128, 2, 4], f32, tag="ps1", bufs=6,
                                   name=f"sufp{t}")
                    for h in range(2):
                        for blk in range(4):
                            nc.tensor.matmul(
                                sufp[:, h, blk:blk + 1],
                                vab[:, t * 4 + blk, h, :],
                                ones_b[:],
                                start=True, stop=True,
                            )
                    c4 = t % 4
                    nc.vector.tensor_copy(
                        colsum_sb[:, :, c4 * 4:c4 * 4 + 4], sufp[:]
                    )

                filler.extend([
                    qk_mms(0, "q", 0), qk_mms(0, "q", 4),
                    q_copy,
                    qk_mms(1, "k", 0), qk_mms(1, "k", 4),
                    k_copy,
                    v_mms(0), v_mms(4),
                    v_copy,
                    transposes(0, 0), transposes(1, 0),
                    va_copies,
                    colsums,
                ])

            # attention for one (batch, chunk): heads interleaved, exp
            # consumers software-pipelined two units behind their producers
            def attn_chunk(b, c, ysb_ch):
                ctok = slice(b * _T + c * _TC, b * _T + (c + 1) * _TC)
                y_ps = {}
                for h in range(2):
                    y_ps[h] = pp.tile([128, _TC], f32, tag="y", bufs=2,
                                      name=f"yps{h}")
                pending = deque()

                def flush(n):
                    while len(pending) > n:
                        pending.popleft()()

                for p in range(0 if V_NOFULL else 2 * c):
                    for h in range(2):
                        rows = slice(h * 64, (h + 1) * 64)
                        pex8 = sp.tile([128, 2, _TC], fp8, tag="pex8",
                                       bufs=12)
                        for par in range(2):
                            kb = 2 * p + par
                            sc = pp.tile([128, _TC], f32, tag="ps1",
                                         bufs=6, name=f"sc{h}{par}")
                            nc.tensor.matmul(
                                sc[:],
                                qk[rows, 1,
                                   b * _T + kb * 128:b * _T + (kb + 1) * 128],
                                qk[rows, 0, ctok],
                                start=True, stop=True,
                            )
                            nc.scalar.activation(
                                pex8[:, par, :], sc[:], AF.Exp,
                                bias=ebias[:], scale=0.125
                            )

                        def dr(p=p, h=h, pex8=pex8):
                            if V_NODR:
                                for par in range(2):
                                    nc.tensor.matmul(
                                        y_ps[h][:],
                                        va8[:, b * _NKB // 2 + p, par, h, :],
                                        pex8[:, par, :],
                                        start=(p == 0 and par == 0),
                                        stop=False,
                                    )
                            else:
                                nc.tensor.matmul(
                                    y_ps[h][:],
                                    va8[:, b * _NKB // 2 + p, :, h, :],
                                    pex8[:],
                                    start=(p == 0),
                                    stop=(V_NOBAND and p == 2 * c - 1),
                                    perf_mode=DR,
                                )
                        pending.append(dr)
                        flush(5)
                        pop_filler(1)
                # band blocks (contain the diagonal): bf16, tail-only.
                # pexb slots: 0 = j0 (full 512), 1 = j1 tail at [128:512],
                # 2 = j2 tail at [0:256] + j3 tail at [256:384]
                pexb = {}
                for h in range(2):
                    pexb[h] = sp.tile([128, 3, _TC], bf16, tag="pexb",
                                      bufs=6, name=f"pexb{h}")
                qbase = b * _T + c * _TC
                for h in range(2 if not V_NOBAND else 0):
                    rows = slice(h * 64, (h + 1) * 64)
                    for par in range(2):
                        j = par
                        off = 128 * j
                        kb = 4 * c + j
                        bd = pp.tile([128, _TC], f32, tag="ps1",
                                     bufs=6, name=f"bd{h}{par}")
                        nc.tensor.matmul(
                            bd[:, off:],
                            qk[rows, 1,
                               b * _T + kb * 128:b * _T + (kb + 1) * 128],
                            qk[rows, 0, qbase + off:qbase + _TC],
                            start=True, stop=True,
                        )
                        nc.scalar.activation(
                            pexb[h][:, j, off:], bd[:, off:],
                            AF.Exp, bias=ebias[:], scale=0.125
                        )
                    flush(5)
                    pop_filler(1)
                for h in range(2 if not V_NOBAND else 0):
                    rows = slice(h * 64, (h + 1) * 64)
                    bd2 = pp.tile([128, _TC], f32, tag="ps1",
                                  bufs=6, name=f"bd2{h}")
                    nc.tensor.matmul(
                        bd2[:, 0:256],
                        qk[rows, 1,
                           b * _T + (4 * c + 2) * 128:
                           b * _T + (4 * c + 3) * 128],
                        qk[rows, 0, qbase + 256:qbase + _TC],
                        start=True, stop=True,
                    )
                    nc.tensor.matmul(
                        bd2[:, 256:384],
                        qk[rows, 1,
                           b * _T + (4 * c + 3) * 128:
                           b * _T + (4 * c + 4) * 128],
                        qk[rows, 0, qbase + 384:qbase + _TC],
                        start=True, stop=True,
                    )
                    nc.scalar.activation(
                        pexb[h][:, 2, 0:384], bd2[:, 0:384],
                        AF.Exp, bias=ebias[:], scale=0.125
                    )
                    flush(4)
                    pop_filler(1)
                flush(0)
                # (slot, col-offset, tail-width, y-offset) per band block j
                band_info = [(0, 0, _TC, 0), (1, 128, 384, 128),
                             (2, 0, 256, 256), (2, 256, 128, 384)]
                for j in range(4 if not V_NOBAND else 0):
                    slot, co, w, yo = band_info[j]
                    for h in range(2):
                        if not V_NOSEL:
                            nc.gpsimd.affine_select(
                                out=pexb[h][:, slot, co:co + 128],
                                in_=pexb[h][:, slot, co:co + 128],
                                compare_op=mybir.AluOpType.is_ge,
                                fill=_EXPM,
                                base=0,
                                pattern=[[1, 128]],
                                channel_multiplier=-1,
                            )

                        def bmm(j=j, h=h, slot=slot, co=co, w=w, yo=yo):
                            nc.tensor.matmul(
                                y_ps[h][:, yo:],
                                vab[:, b * _NKB + 4 * c + j, h, :],
                                pexb[h][:, slot, co:co + w],
                                start=((c == 0 or V_NOFULL) and j == 0),
                                stop=(j == 3),
                            )
                        pending.append(bmm)
                        flush(4)
                        pop_filler(1)
                while pending:
                    pop_filler(1)
                    pending.popleft()()

                def stage(h):
                    def go():
                        if V_NOBAND:
                            return
                        ysb = sp.tile([128, _TC], bf16, tag="ysb", bufs=10,
                                      name=f"ysb{h}")
                        nc.vector.tensor_copy(ysb[:], y_ps[h][:])
                        ysb_ch[(c, h)] = ysb
                    return go
                filler.appendleft(stage(1))
                filler.appendleft(stage(0))
                pop_filler(2)

            def scan_suffix(colsum_sb, h, xs_out):
                def go():
                    if V_NOCS:
                        return
                    xa_t = sp.tile([128, 24], f32, tag="sfx", bufs=2)
                    xb_t = sp.tile([128, 24], f32, tag="sfy", bufs=2)
                    nc.vector.memset(xa_t[:, 16:24], 0.0)
                    nc.vector.memset(xb_t[:, 16:24], 0.0)
                    nc.vector.tensor_copy(xa_t[:, 0:16],
                                          colsum_sb[:, h, 0:16])
                    nc.vector.tensor_add(xb_t[:, 0:16], xa_t[:, 0:16],
                                         xa_t[:, 1:17])
                    nc.vector.tensor_add(xa_t[:, 0:16], xb_t[:, 0:16],
                                         xb_t[:, 2:18])
                    nc.vector.tensor_add(xb_t[:, 0:16], xa_t[:, 0:16],
                                         xa_t[:, 4:20])
                    nc.vector.tensor_add(xa_t[:, 0:16], xb_t[:, 0:16],
                                         xb_t[:, 8:24])
                    xs = sp.tile([128, 17], f32, tag="xs", bufs=4,
                                 name=f"xs{h}")
                    nc.vector.tensor_scalar_mul(xs[:], xa_t[:, 0:17], _EXPM)
                    xs_out[h] = xs
                return go

            def emit_normalize(b, ysb_ch, xs_out, ycat):
                def norm(c, h):
                    def go():
                        if V_NONORM:
                            return
                        ysb = ysb_ch[(c, h)]
                        for g in range(4):
                            qb = 4 * c + g
                            nc.vector.tensor_scalar_add(
                                ysb[:, g * 128:(g + 1) * 128],
                                ysb[:, g * 128:(g + 1) * 128],
                                xs_out[h][:, qb + 1:qb + 2],
                            )
                        zr = sp.tile([64, _TC], bf16, tag="zr", bufs=4)
                        with nc.allow_low_precision(
                            reason="z in bf16; rel tolerance is 2e-2"
                        ):
                            nc.vector.reciprocal(zr[:], ysb[64:128, :])
                        nc.vector.tensor_mul(
                            ycat[h * 64:(h + 1) * 64,
                                 c * _TC:(c + 1) * _TC],
                            ysb[0:64, :], zr[:]
                        )
                    return go
                for c in range(_NQC):
                    for h in range(2):
                        filler.append(norm(c, h))

            def emit_outproj(b, ycat):
                def munit(m):
                    def go():
                        if V_NOOP:
                            return
                        ostg = sp.tile([128, 4, _TC], bf16, tag="ostg",
                                       bufs=3)
                        for c2 in range(4):
                            ops = pp.tile([128, _TC], f32, tag="ps1",
                                          bufs=6)
                            nc.tensor.matmul(
                                ops[:],
                                wo_sb[:, m, :],
                                ycat[:, c2 * _TC:(c2 + 1) * _TC],
                                start=True, stop=True,
                            )
                            if c2 == 0:
                                nc.scalar.copy(ostg[:, c2, :], ops[:])
                            else:
                                nc.vector.tensor_copy(
                                    ostg[:, c2, :], ops[:]
                                )
                        nc.sync.dma_start(
                            outT_d[m * 128:(m + 1) * 128,
                                   b * _T:(b + 1) * _T],
                            ostg[:],
                        )
                    return go
                for m in range(_KC):
                    filler.append(munit(m))

            # ---------------- main schedule: flat pipeline ----------------
            state = {}

            def make_state(b2):
                nc.gpsimd.memset(
                    vab[:, b2 * _NKB:(b2 + 1) * _NKB, :, 64:128], 1.0)
                nc.gpsimd.memset(
                    va8[:, b2 * _NKB // 2:(b2 + 1) * _NKB // 2, :, :,
                        64:128], 1.0)
                state[b2] = {
                    "colsum": sp.tile([128, 2, 16], f32, tag="colsum",
                                      bufs=2, name=f"colsum{b2}"),
                    "ysb": {},
                    "xs": {},
                    "ycat": sp.tile([128, _T], bf16, tag="ycat",
                                    bufs=2, name=f"ycat{b2}"),
                }

            make_state(0)
            for t in range(_NTC):
                b, c = divmod(t, 4)
                if t == 0:
                    emit_proj(0, state[0]["colsum"])
                    drain_filler()
                # queue next chunk's projection work as filler
                if t + 1 < _NTC:
                    b2, c2 = divmod(t + 1, 4)
                    if c2 == 0:
                        make_state(b2)
                    emit_proj(t + 1, state[b2]["colsum"])
                if c == 3:
                    filler.append(scan_suffix(state[b]["colsum"], 0,
                                              state[b]["xs"]))
                    filler.append(scan_suffix(state[b]["colsum"], 1,
                                              state[b]["xs"]))
                attn_chunk(b, c, state[b]["ysb"])
                if c == 3:
                    emit_normalize(b, state[b]["ysb"], state[b]["xs"],
                                   state[b]["ycat"])
                if c == 1 and b > 0:
                    emit_outproj(b - 1, state[b - 1]["ycat"])
            drain_filler()
            emit_outproj(_B - 1, state[_B - 1]["ycat"])
            drain_filler()

    nc.compile()
    return nc, outT_d.name


def _get_nc():
    if "nc" not in _cache:
        _cache["nc"] = _build()
    return _cache["nc"]


def kernel(**inputs):
    import ml_dtypes

    from concourse.bass_utils import run_bass_kernel_spmd

    x = np.ascontiguousarray(np.asarray(inputs["x"]), dtype=np.float32)
    wq = np.ascontiguousarray(np.asarray(inputs["wq"]), dtype=np.float32)
    wk = np.ascontiguousarray(np.asarray(inputs["wk"]), dtype=np.float32)
    wv = np.ascontiguousarray(np.asarray(inputs["wv"]), dtype=np.float32)
    wo = np.ascontiguousarray(np.asarray(inputs["wo"]), dtype=np.float32)

    bf16 = ml_dtypes.bfloat16
    xT = np.ascontiguousarray(x.reshape(_BT, _C).T).astype(bf16)
    identc = np.zeros((128, 64), dtype=np.float32)
    identc[np.arange(128), np.arange(128) % 64] = 1.0
    onesb = np.ones((128, 1), dtype=bf16)

    in_maps = []
    for i in range(_NC):
        r = slice(_LOC * i, _LOC * (i + 1))
        in_maps.append({
            "xT": xT,
            "wqT": np.ascontiguousarray(wq[r].T).astype(bf16),
            "wkT": np.ascontiguousarray(wk[r].T).astype(bf16),
            "wvT": np.ascontiguousarray(wv[r].T).astype(bf16),
            "woT": np.ascontiguousarray(wo[:, r].T).astype(bf16),
            "identc": identc,
            "onesb": onesb,
        })

    nc, outname = _get_nc()
    try:
        res = run_bass_kernel_spmd(nc, in_maps, list(range(_NC)), trace=TRACE)
    except ModuleNotFoundError:
        # NTFF profiling hook unavailable in this container
        res = run_bass_kernel_spmd(nc, in_maps, list(range(_NC)), trace=False)

    global LAST_EXEC_NS, LAST_RESULTS
    LAST_EXEC_NS = res.exec_time_ns
    LAST_RESULTS = res

    acc = np.zeros((_C, _BT), dtype=np.float64)
    for i in range(_NC):
        acc += res.results[i][outname].astype(np.float64)
    return np.ascontiguousarray(acc.T).reshape(_B, _T, _C).astype(np.float32)
